# revision 12
# baseline (speedup 1.0000x reference)
"""GCN v3: v1's dma_gather data path + v2's overlapped exchange.

- Window-pair buckets (minimal chunk count; gather is the Q7-bound cost).
- Group-major table rows, double-buffered across layers, so each window
  group's slice is AllGather'd as soon as its next-layer transform is done,
  overlapping the tail of the current layer's aggregation.
- conv1's table is built through the same per-group exchange (each core
  transforms only its own slice).
"""

import math
import sys

sys.path.insert(0, "/opt/trn_rl_repo")

import numpy as np
import ml_dtypes

import concourse.bass as bass
import concourse.mybir as mybir
import concourse.tile as tile
from concourse import bacc
from concourse.masks import make_identity

BF16 = mybir.dt.bfloat16
F32 = mybir.dt.float32
I16 = mybir.dt.int16
ALU = mybir.AluOpType

NP_BF16 = ml_dtypes.bfloat16

CALLCH = 48        # chunks per dma_gather call (6144 idxs, multi-packet;
                   # bufs*CALLCH*128 descs stays under the 16384-desc carveout)
SBATCH = 8         # chunks per S-build DVE op
WGRP = 16          # dst windows per PSUM accumulation group


def _ap3(ap, pattern, offset=None):
    return bass.AP(ap.tensor, ap.offset if offset is None else offset, pattern)


def prep(x, W1, b1, W2, b2, Wl, bl, edge_index, batch, C, G):
    x = np.asarray(x, np.float32)
    W1 = np.asarray(W1, np.float32); b1 = np.asarray(b1, np.float32)
    W2 = np.asarray(W2, np.float32); b2 = np.asarray(b2, np.float32)
    Wl = np.asarray(Wl, np.float32); bl = np.asarray(bl, np.float32)
    edge_index = np.asarray(edge_index, np.int64)
    batch = np.asarray(batch, np.int64)

    N, F = x.shape
    E = edge_index.shape[1]
    H = W1.shape[1]
    assert N % C == 0 and C % 2 == 0
    NPC = N // C
    W = math.ceil(NPC / 128)
    NPAD = W * 128
    ROWS = C * NPAD
    HR = ROWS // 2
    assert HR <= 32768, HR
    NG = math.ceil(W / WGRP)
    nbs = [min(WGRP, W - g * WGRP) for g in range(NG)]
    cum_nb = np.concatenate([[0], np.cumsum(nbs)])
    base_g = [int(C * 128 * cum_nb[g]) for g in range(NG)]

    src, dst = edge_index[0], edge_index[1]
    deg = 1.0 + np.bincount(dst, minlength=N).astype(np.float32)
    dis = 1.0 / np.sqrt(deg)

    n = np.arange(N)
    cb = n // NPC
    lp = n % NPC
    p_ = lp % 128
    w_ = lp // 128
    g_ = w_ // WGRP
    wl_ = w_ - g_ * WGRP
    nb_ = np.array(nbs)[g_]
    srow = np.array(base_g)[g_] + cb * (nb_ * 128) + p_ * nb_ + wl_
    xcol = cb * NPAD + w_ * 128 + p_

    # --- edge bucketing: key = (src half, dst window pair) -----------------
    ecore = dst // NPC
    edl = dst % NPC
    ewp = edl // 256            # window pair
    edloc = edl % 256           # dst slot within the pair
    esh = (srow[src] >= HR).astype(np.int64)
    WP = math.ceil(W / 2)
    PPG = WGRP // 2             # pairs per psum group

    cnt = np.zeros((C, 2, WP), np.int64)
    np.add.at(cnt, (ecore, esh, ewp), 1)
    Kb = np.ceil(cnt.max(axis=0) / 128).astype(np.int64)
    for wp in range(WP):
        if Kb[:, wp].sum() == 0:
            Kb[0, wp] = 1

    # chunk order: (group, src half, window pair)
    chunk_w, chunk_sh = [], []
    seg_bounds = []
    boff = np.zeros((2, WP), np.int64)
    for g in range(NG):
        plo, phi = g * PPG, min((g + 1) * PPG, WP)
        for sh in (0, 1):
            lo = len(chunk_w)
            for wp in range(plo, phi):
                boff[sh, wp] = len(chunk_w)
                for _ in range(int(Kb[sh, wp])):
                    chunk_w.append(wp); chunk_sh.append(sh)
            if len(chunk_w) > lo:
                seg_bounds.append((sh, lo, len(chunk_w)))
    NCHUNK = len(chunk_w)
    chunk_w = np.array(chunk_w); chunk_sh = np.array(chunk_sh)

    bank_of_chunk = (2 * chunk_w) // 8
    start_f = np.zeros(NCHUNK, bool); stop_f = np.zeros(NCHUNK, bool)
    for b in np.unique(bank_of_chunk):
        idxs = np.nonzero(bank_of_chunk == b)[0]
        start_f[idxs[0]] = True; stop_f[idxs[-1]] = True

    calls = []
    for sh, lo, hi in seg_bounds:
        c0 = lo
        while c0 < hi:
            c1 = min(c0 + CALLCH, hi)
            calls.append((sh, c0, c1))
            c0 = c1
    grp_of_chunk = (2 * chunk_w) // WGRP

    # --- per-core edge payloads --------------------------------------------
    idx_all = np.zeros((C, NCHUNK * 128), np.int16)
    dl_all = np.full((C, NCHUNK * 128), 400.0, np.float32)
    for c in range(C):
        m = ecore == c
        es, ish, iw, idl = src[m], esh[m], ewp[m], edloc[m]
        order = np.lexsort((srow[es], iw, ish))
        es, ish, iw, idl = es[order], ish[order], iw[order], idl[order]
        key = ish * WP + iw
        uniq, first = np.unique(key, return_index=True)
        ranks = np.arange(len(key)) - first[np.searchsorted(uniq, key)]
        pos = boff[ish, iw] * 128 + ranks
        idx_all[c, pos] = (srow[es] - ish * HR).astype(np.int16)
        dl_all[c, pos] = idl

    idx16 = np.zeros((C, 128, NCHUNK * 8), np.int16)
    for c in range(C):
        wrapped = idx_all[c].reshape(NCHUNK * 8, 16).T
        idx16[c] = np.tile(wrapped, (8, 1))
    dstloc = np.zeros((C, 128, NCHUNK), NP_BF16)
    for c in range(C):
        dstloc[c] = dl_all[c].reshape(NCHUNK, 128).T.astype(NP_BF16)

    # --- node-side tensors --------------------------------------------------
    xfm = np.zeros((F, ROWS), np.float32)
    xfm[:, xcol] = x.T
    xfm = xfm.astype(NP_BF16)

    disn = np.zeros((C, 128, W), np.float32)
    for c in range(C):
        sl = slice(c * NPC, (c + 1) * NPC)
        disn[c][p_[sl], w_[sl]] = dis[sl]

    tailp = NPC - (W - 1) * 128
    mask48 = (np.arange(128) < tailp).astype(np.float32).reshape(128, 1)

    BLK = math.ceil(G / 128) + 3
    wk = [int(batch[c * NPC]) // 128 for c in range(C)]
    glocal = np.full((C, 128, W), 1.0e4, np.float32)
    for c in range(C):
        sl = slice(c * NPC, (c + 1) * NPC)
        gl = batch[sl] - 128 * wk[c]
        assert gl.min() >= 0 and gl.max() < 384, (c, gl.min(), gl.max())
        glocal[c][p_[sl], w_[sl]] = gl

    meta = dict(
        N=N, F=F, H=H, E=E, G=G, C=C, NPC=NPC, W=W, NPAD=NPAD, ROWS=ROWS,
        HR=HR, NG=NG, nbs=nbs, cum_nb=cum_nb, base_g=base_g,
        NCHUNK=NCHUNK, chunk_w=chunk_w, chunk_sh=chunk_sh,
        bank_of_chunk=bank_of_chunk,
        start_f=start_f, stop_f=stop_f, calls=calls, grp_of_chunk=grp_of_chunk,
        wk=wk, BLK=BLK, tailp=tailp,
    )

    shared = dict(
        W1sb=W1.astype(NP_BF16),
        W2sb=W2.astype(NP_BF16),
        b1sb=np.tile(b1.reshape(1, H), (128, 1)).astype(np.float32),
        b2sb=np.tile(b2.reshape(1, H), (128, 1)).astype(np.float32),
        iota128=np.tile(np.arange(256, dtype=np.float32).reshape(1, 256),
                        (128, 1)).astype(NP_BF16),
        iotaP=np.tile(np.arange(384, dtype=np.float32).reshape(1, 384),
                      (128, 1)),
        mask48=mask48,
    )
    in_maps = []
    for c in range(C):
        m = dict(shared)
        m["xown"] = np.ascontiguousarray(xfm[:, c * NPAD:(c + 1) * NPAD])
        m["idx16"] = idx16[c]
        m["dstloc"] = dstloc[c]
        m["disn"] = disn[c]
        m["glocal"] = glocal[c]
        in_maps.append(m)
    return meta, in_maps


def build(nc, meta):
    F, H, C = meta["F"], meta["H"], meta["C"]
    W, NPAD, ROWS, HR = meta["W"], meta["NPAD"], meta["ROWS"], meta["HR"]
    NG, NCHUNK, BLK, G = meta["NG"], meta["NCHUNK"], meta["BLK"], meta["G"]
    nbs, cum_nb, base_g = meta["nbs"], meta["cum_nb"], meta["base_g"]
    chunk_w = meta["chunk_w"]
    start_f, stop_f = meta["start_f"], meta["stop_f"]
    calls, wk = meta["calls"], meta["wk"]
    rg = [list(range(C))]

    W1_e = nc.dram_tensor("W1sb", [F, H], BF16, kind="ExternalInput")
    W2_e = nc.dram_tensor("W2sb", [H, H], BF16, kind="ExternalInput")
    b1_e = nc.dram_tensor("b1sb", [128, H], F32, kind="ExternalInput")
    b2_e = nc.dram_tensor("b2sb", [128, H], F32, kind="ExternalInput")
    iota_e = nc.dram_tensor("iota128", [128, 256], BF16, kind="ExternalInput")
    iotaP_e = nc.dram_tensor("iotaP", [128, 384], F32, kind="ExternalInput")
    mask_e = nc.dram_tensor("mask48", [128, 1], F32, kind="ExternalInput")
    idx_e = nc.dram_tensor("idx16", [128, NCHUNK * 8], I16, kind="ExternalInput")
    dstloc_e = nc.dram_tensor("dstloc", [128, NCHUNK], BF16, kind="ExternalInput")
    disn_e = nc.dram_tensor("disn", [128, W], F32, kind="ExternalInput")
    glocal_e = nc.dram_tensor("glocal", [128, W], F32, kind="ExternalInput")
    xown_e = nc.dram_tensor("xown", [F, NPAD], BF16, kind="ExternalInput")
    out_e = nc.dram_tensor("out", [128, 3 * H], F32, kind="ExternalOutput")

    PW = 3 * H
    shared = "Shared" if C > 4 else "Local"
    tblA = nc.dram_tensor("tblA", [ROWS, 128], BF16, addr_space=shared)
    tblB = nc.dram_tensor("tblB", [ROWS, 128], BF16, addr_space=shared)
    own_slice = nc.dram_tensor("own_slice", [NPAD, 128], BF16)

    with tile.TileContext(nc) as tc:
        from contextlib import ExitStack
        with ExitStack() as ctx:
            cpool = ctx.enter_context(tc.tile_pool(name="const", bufs=1))
            spool = ctx.enter_context(tc.tile_pool(name="s", bufs=8))
            mpool = ctx.enter_context(tc.tile_pool(name="msg", bufs=2))
            tpool = ctx.enter_context(tc.tile_pool(name="tmp", bufs=3))
            agg_ps = ctx.enter_context(
                tc.tile_pool(name="aggps", bufs=2, space="PSUM"))
            tp_ps = ctx.enter_context(
                tc.tile_pool(name="tpps", bufs=2, space="PSUM"))

            def load(name, ext, shape, dt):
                t = cpool.tile(shape, dt, tag=name)
                nc.sync.dma_start(t[:], ext.ap())
                return t

            W1sb = load("W1", W1_e, [F, H], BF16)
            disn = load("disn", disn_e, [128, W], F32)
            W2sb = load("W2", W2_e, [H, H], BF16)
            b1sb = load("b1", b1_e, [128, H], F32)
            b2sb = load("b2", b2_e, [128, H], F32)
            iota = load("iota", iota_e, [128, 256], BF16)
            iotaP = load("iotaP", iotaP_e, [128, 384], F32)
            mask48 = load("mask48", mask_e, [128, 1], F32)
            idxsb = load("idx", idx_e, [128, NCHUNK * 8], I16)
            dstloc = load("dstloc", dstloc_e, [128, NCHUNK], BF16)
            glocal = load("glocal", glocal_e, [128, W], F32)
            ident = cpool.tile([128, 128], BF16, tag="ident")
            make_identity(nc, ident[:])

            h_fm = [cpool.tile([H, NPAD], BF16, tag=f"hfm{i}", name=f"hfm{i}")
                    for i in range(2)]
            t_own = cpool.tile([128, W * H], BF16, tag="town")
            tpad = cpool.tile([128, W * 128], BF16, tag="tpad")
            nc.vector.memset(tpad[:], 0.0)
            h_nm = cpool.tile([128, W * H], BF16, tag="hnm")

            def iota_bc(nb, width=256, base=0):
                a = iota[:]
                return _ap3(a, [[a.ap[0][0], 128], [0, nb], [1, width]], base)

            def sc_bc(t, lo, n, inner):
                a = t[:]
                step = a.ap[0][0]
                return _ap3(a, [[step, 128], [1, n], [0, inner]], a.offset + lo)

            def transform_own_grp(h_src, Wsb, g):
                wlo = g * WGRP
                nb = nbs[g]
                ps = agg_ps.tile([128, WGRP * H], F32, tag="agg",
                                 name=f"tf{g}")
                for i in range(nb):
                    w = wlo + i
                    nc.tensor.matmul(
                        ps[:, i * H:(i + 1) * H],
                        lhsT=h_src[:, w * 128:(w + 1) * 128],
                        rhs=Wsb[:],
                        start=True, stop=True, skip_group_check=True)
                ps3 = _ap3(ps[:], [[ps[:].ap[0][0], 128], [H, nb], [1, H]])
                t3 = _ap3(t_own[:], [[t_own[:].ap[0][0], 128], [H, nb], [1, H]],
                          t_own[:].offset + wlo * H)
                nc.vector.tensor_tensor(
                    t3, ps3, sc_bc(disn, wlo, nb, H), op=ALU.mult)
                tp3 = _ap3(tpad[:], [[tpad[:].ap[0][0], 128], [128, nb], [1, H]],
                           tpad[:].offset + wlo * 128)
                to3 = _ap3(t_own[:], [[t_own[:].ap[0][0], 128], [H, nb], [1, H]],
                           t_own[:].offset + wlo * H)
                nc.vector.tensor_copy(tp3, to3)

            pending_cc = []

            def exchange_grp(g, tbl, defer=False):
                nb = nbs[g]
                wlo = g * WGRP
                osl = _ap3(own_slice.ap(),
                           [[nb * 128, 128], [1, nb * 128]],
                           int(cum_nb[g]) * 128 * 128)
                nc.sync.dma_start(osl, tpad[:, wlo * 128:(wlo + nb) * 128])
                ins = bass.AP(own_slice.ap().tensor,
                              int(cum_nb[g]) * 128 * 128,
                              [[1, nb * 128 * 128]])
                outs = bass.AP(tbl.ap().tensor, base_g[g] * 128,
                               [[1, C * nb * 128 * 128]])
                if defer:
                    pending_cc.append((ins, outs))
                    return
                nc.gpsimd.collective_compute(
                    "AllGather", ALU.bypass, replica_groups=rg,
                    ins=[ins.opt()], outs=[outs.opt()])

            def flush_cc():
                while pending_cc:
                    ins, outs = pending_cc.pop(0)
                    nc.gpsimd.collective_compute(
                        "AllGather", ALU.bypass, replica_groups=rg,
                        ins=[ins.opt()], outs=[outs.opt()])

            def aggregate_and_drain(layer, bsb, tbl, after_group=None):
                psg = {}
                bank_start = {}
                glast = {}
                for ci in range(NCHUNK):
                    glast[int(meta["grp_of_chunk"][ci])] = ci
                drain_after = {v: k for k, v in glast.items()}
                for ci_call, (sh, c_lo, c_hi) in enumerate(calls):
                    if ci_call >= 1:
                        flush_cc()
                    ncall = c_hi - c_lo
                    msg = mpool.tile([128, CALLCH * 128], BF16, tag="msg")
                    in_ap = bass.AP(tbl.ap().tensor, sh * HR * 128,
                                    [[128, HR], [1, 128]])
                    nc.gpsimd.dma_gather(
                        out_ap=_ap3(msg[:], [[msg[:].ap[0][0], 128],
                                             [128, ncall], [1, 128]]),
                        in_ap=in_ap,
                        idxs_ap=idxsb[:, c_lo * 8:c_hi * 8],
                        num_idxs=ncall * 128,
                        num_idxs_reg=ncall * 128,
                        elem_size=128,
                        single_packet=(ncall * 128 <= 1024))
                    for s0 in range(0, ncall, SBATCH):
                        nb_s = min(SBATCH, ncall - s0)
                        S = spool.tile([128, SBATCH * 256], BF16, tag="S")
                        S3 = _ap3(S[:], [[S[:].ap[0][0], 128], [256, nb_s],
                                         [1, 256]])
                        nc.vector.tensor_tensor(
                            S3, iota_bc(nb_s),
                            sc_bc(dstloc, c_lo + s0, nb_s, 256),
                            op=ALU.is_equal)
                        for j in range(nb_s):
                            ci = c_lo + s0 + j
                            wp = int(chunk_w[ci])
                            wA = 2 * wp
                            g = wA // WGRP
                            if g not in psg:
                                psg[g] = agg_ps.tile(
                                    [128, WGRP * H], F32, tag="agg",
                                    name=f"agg_l{layer}_g{g}")
                            wl = wA - g * WGRP
                            has_b = wA + 1 < W
                            rhs = msg[:, (s0 + j) * 128:(s0 + j) * 128 + H]
                            mm = nc.tensor.matmul(
                                psg[g][:, wl * H:(wl + 1) * H],
                                lhsT=S[:, j * 256:j * 256 + 128],
                                rhs=rhs,
                                start=bool(start_f[ci]),
                                stop=bool(stop_f[ci]) and not has_b,
                                skip_group_check=True)
                            bk = int(meta["bank_of_chunk"][ci])
                            if start_f[ci]:
                                bank_start[bk] = mm
                            elif bk in bank_start:
                                bass._add_dep_helper(
                                    mm.ins, bank_start[bk].ins, sync=False,
                                    reason="psum zero-region order")
                            if has_b:
                                mmb = nc.tensor.matmul(
                                    psg[g][:, (wl + 1) * H:(wl + 2) * H],
                                    lhsT=S[:, j * 256 + 128:(j + 1) * 256],
                                    rhs=rhs,
                                    start=False, stop=bool(stop_f[ci]),
                                    skip_group_check=True)
                                bass._add_dep_helper(
                                    mmb.ins, bank_start[bk].ins, sync=False,
                                    reason="psum zero-region order")
                            if ci in drain_after:
                                gdone = drain_after[ci]
                                drain_group(gdone, psg.pop(gdone), bsb)
                                if after_group is not None:
                                    after_group(gdone)

            def drain_group(g, ps, bsb):
                wlo = g * WGRP
                nb = nbs[g]
                pstep = ps[:].ap[0][0]
                ps3 = _ap3(ps[:], [[pstep, 128], [H, nb], [1, H]])
                tmp = tpool.tile([128, WGRP * H], F32, tag="dr")
                ts = tmp[:].ap[0][0]
                tmp3 = _ap3(tmp[:], [[ts, 128], [H, nb], [1, H]])
                to3 = _ap3(t_own[:], [[t_own[:].ap[0][0], 128], [H, nb], [1, H]],
                           t_own[:].offset + wlo * H)
                nc.vector.tensor_tensor(tmp3, ps3, to3, op=ALU.add)
                nc.vector.tensor_tensor(
                    tmp3, tmp3, sc_bc(disn, wlo, nb, H), op=ALU.mult)
                bb = _ap3(bsb[:], [[bsb[:].ap[0][0], 128], [0, nb], [1, H]], 0)
                nc.vector.tensor_tensor(tmp3, tmp3, bb, op=ALU.add)
                hn3 = _ap3(h_nm[:], [[h_nm[:].ap[0][0], 128], [H, nb], [1, H]],
                           h_nm[:].offset + wlo * H)
                nc.vector.tensor_scalar(hn3, tmp3, 0.0, None, op0=ALU.max)
                if g == NG - 1 and meta["tailp"] < 128:
                    lastw = W - 1
                    hl = _ap3(h_nm[:], [[h_nm[:].ap[0][0], 128], [1, H]],
                              h_nm[:].offset + lastw * H)
                    mb = _ap3(mask48[:],
                              [[mask48[:].ap[0][0], 128], [0, H]], 0)
                    nc.vector.tensor_tensor(hl, hl, mb, op=ALU.mult)

            def to_fm_grp(dst_fm, g):
                for w in range(g * WGRP, g * WGRP + nbs[g]):
                    tp = tp_ps.tile([H, 128], BF16, tag="tp")
                    nc.tensor.transpose(
                        out=tp[:],
                        in_=_ap3(h_nm[:], [[h_nm[:].ap[0][0], 128], [1, H]],
                                 h_nm[:].offset + w * H),
                        identity=ident[:])
                    nc.scalar.copy(dst_fm[:, w * 128:(w + 1) * 128], tp[:])

            # ================= layer schedule =================
            tbls = [tblA, tblB]
            with tc.tile_pool(name="xfm", bufs=2) as xpool:
                xo = xpool.tile([F, NPAD], BF16, tag="xfm", name="xo")
                nc.sync.dma_start(xo[:], xown_e.ap())
                for g in range(NG):
                    transform_own_grp(xo, W1sb, g)
                    exchange_grp(g, tbls[0])

            pool_state = {}

            def pooling_grp(g):
                wlo = g * WGRP
                nwg = nbs[g]
                pps = pool_state["pps"]
                for blk in range(3):
                    for w0 in range(wlo, wlo + nwg, SBATCH):
                        nb = min(SBATCH, wlo + nwg - w0)
                        SG = spool.tile([128, SBATCH * 256], BF16, tag="S")
                        iob = _ap3(iotaP[:], [[iotaP[:].ap[0][0], 128],
                                              [0, nb], [1, 128]], blk * 128)
                        nc.vector.tensor_tensor(
                            _ap3(SG[:], [[SG[:].ap[0][0], 128], [128, nb],
                                         [1, 128]]),
                            iob, sc_bc(glocal, w0, nb, 128), op=ALU.is_equal)
                        for i in range(nb):
                            w = w0 + i
                            mm = nc.tensor.matmul(
                                pps[:, blk * H:(blk + 1) * H],
                                lhsT=SG[:, i * 128:(i + 1) * 128],
                                rhs=_ap3(h_nm[:],
                                         [[h_nm[:].ap[0][0], 128], [1, H]],
                                         h_nm[:].offset + w * H),
                                start=(blk == 0 and w == 0),
                                stop=(blk == 2 and w == W - 1),
                                skip_group_check=True)
                            if blk == 0 and w == 0:
                                pool_state["start"] = mm
                            else:
                                bass._add_dep_helper(
                                    mm.ins, pool_state["start"].ins,
                                    sync=False,
                                    reason="psum zero-region order")

            for l in range(1, 6):
                bsb = b1sb if l == 1 else b2sb
                tbl_r = tbls[(l - 1) % 2]
                tbl_w = tbls[l % 2]
                if l < 5:
                    hf_next = h_fm[(l + 1) % 2]

                    def after_group(g, hf=hf_next, tw=tbl_w):
                        to_fm_grp(hf, g)
                        transform_own_grp(hf, W2sb, g)
                        exchange_grp(g, tw, defer=True)
                else:
                    pool_state["pps"] = agg_ps.tile(
                        [128, WGRP * H], F32, tag="agg", name="pps")

                    def after_group(g):
                        pooling_grp(g)
                aggregate_and_drain(l, bsb, tbl_r, after_group=after_group)

            flush_cc()
            ppsb = tpool.tile([128, PW], F32, tag="ppsb")
            nc.vector.tensor_copy(ppsb[:], pool_state["pps"][:, :PW])
            nc.sync.dma_start(out_e.ap(), ppsb[:])


def run(inputs, C=8, G=1000, trace=False):
    meta, in_maps = prep(
        inputs["x"], inputs["W1"], inputs["b1"], inputs["W2"], inputs["b2"],
        inputs["Wl"], inputs["bl"], inputs["edge_index"], inputs["batch"],
        C=C, G=G)
    nc = bacc.Bacc("TRN2", target_bir_lowering=False, debug=False,
                   num_devices=C)
    build(nc, meta)
    nc.compile()
    from concourse.bass_utils import run_bass_kernel_spmd
    res = run_bass_kernel_spmd(nc, in_maps, core_ids=list(range(C)),
                               trace=trace)
    parts = [res.results[c]["out"] for c in range(C)]
    out = host_finish(meta, parts, inputs, C, G)
    return out, res


def host_finish(meta, parts, inputs, C, G):
    H = meta["H"]
    pooled = np.zeros(((meta["BLK"] + 3) * 128, H), np.float32)
    for c in range(C):
        part = np.asarray(parts[c], np.float32)
        base = meta["wk"][c] * 128
        for b in range(3):
            pooled[base + b * 128: base + (b + 1) * 128] += \
                part[:, b * H:(b + 1) * H]
    counts = np.bincount(np.asarray(inputs["batch"], np.int64),
                         minlength=G).astype(np.float32)
    pooledG = pooled[:G] / np.maximum(counts, 1.0)[:, None]
    Wl = np.asarray(inputs["Wl"], np.float32).reshape(H, -1)
    bl = np.asarray(inputs["bl"], np.float32)
    return (pooledG @ Wl + bl).astype(np.float32)


def kernel(**inputs):
    out, _ = run(inputs)
    return out


# revision 15
# speedup vs baseline: 1.0070x; 1.0070x over previous
"""GCN v3: v1's dma_gather data path + v2's overlapped exchange.

- Window-pair buckets (minimal chunk count; gather is the Q7-bound cost).
- Group-major table rows, double-buffered across layers, so each window
  group's slice is AllGather'd as soon as its next-layer transform is done,
  overlapping the tail of the current layer's aggregation.
- conv1's table is built through the same per-group exchange (each core
  transforms only its own slice).
"""

import math
import sys

sys.path.insert(0, "/opt/trn_rl_repo")

import numpy as np
import ml_dtypes

import concourse.bass as bass
import concourse.mybir as mybir
import concourse.tile as tile
from concourse import bacc
from concourse.masks import make_identity

BF16 = mybir.dt.bfloat16
F32 = mybir.dt.float32
I16 = mybir.dt.int16
ALU = mybir.AluOpType

NP_BF16 = ml_dtypes.bfloat16

CALLCH = 32        # chunks per dma_gather call (4096 idxs, multi-packet)
SBATCH = 8         # chunks per S-build DVE op
WGRP = 16          # dst windows per PSUM accumulation group


def _ap3(ap, pattern, offset=None):
    return bass.AP(ap.tensor, ap.offset if offset is None else offset, pattern)


def prep(x, W1, b1, W2, b2, Wl, bl, edge_index, batch, C, G):
    x = np.asarray(x, np.float32)
    W1 = np.asarray(W1, np.float32); b1 = np.asarray(b1, np.float32)
    W2 = np.asarray(W2, np.float32); b2 = np.asarray(b2, np.float32)
    Wl = np.asarray(Wl, np.float32); bl = np.asarray(bl, np.float32)
    edge_index = np.asarray(edge_index, np.int64)
    batch = np.asarray(batch, np.int64)

    N, F = x.shape
    E = edge_index.shape[1]
    H = W1.shape[1]
    assert N % C == 0 and C % 2 == 0
    NPC = N // C
    W = math.ceil(NPC / 128)
    NPAD = W * 128
    ROWS = C * NPAD
    HR = ROWS // 2
    assert HR <= 32768, HR
    NG = math.ceil(W / WGRP)
    nbs = [min(WGRP, W - g * WGRP) for g in range(NG)]
    cum_nb = np.concatenate([[0], np.cumsum(nbs)])
    base_g = [int(C * 128 * cum_nb[g]) for g in range(NG)]

    src, dst = edge_index[0], edge_index[1]
    deg = 1.0 + np.bincount(dst, minlength=N).astype(np.float32)
    dis = 1.0 / np.sqrt(deg)

    n = np.arange(N)
    cb = n // NPC
    lp = n % NPC
    p_ = lp % 128
    w_ = lp // 128
    g_ = w_ // WGRP
    wl_ = w_ - g_ * WGRP
    nb_ = np.array(nbs)[g_]
    srow = np.array(base_g)[g_] + cb * (nb_ * 128) + p_ * nb_ + wl_
    xcol = cb * NPAD + w_ * 128 + p_

    # --- edge bucketing: key = (src half, dst window pair) -----------------
    ecore = dst // NPC
    edl = dst % NPC
    ewp = edl // 256            # window pair
    edloc = edl % 256           # dst slot within the pair
    esh = (srow[src] >= HR).astype(np.int64)
    WP = math.ceil(W / 2)
    PPG = WGRP // 2             # pairs per psum group

    cnt = np.zeros((C, 2, WP), np.int64)
    np.add.at(cnt, (ecore, esh, ewp), 1)
    Kb = np.ceil(cnt.max(axis=0) / 128).astype(np.int64)
    for wp in range(WP):
        if Kb[:, wp].sum() == 0:
            Kb[0, wp] = 1

    # chunk order: (group, src half, window pair)
    chunk_w, chunk_sh = [], []
    seg_bounds = []
    boff = np.zeros((2, WP), np.int64)
    for g in range(NG):
        plo, phi = g * PPG, min((g + 1) * PPG, WP)
        for sh in (0, 1):
            lo = len(chunk_w)
            for wp in range(plo, phi):
                boff[sh, wp] = len(chunk_w)
                for _ in range(int(Kb[sh, wp])):
                    chunk_w.append(wp); chunk_sh.append(sh)
            if len(chunk_w) > lo:
                seg_bounds.append((sh, lo, len(chunk_w)))
    NCHUNK = len(chunk_w)
    chunk_w = np.array(chunk_w); chunk_sh = np.array(chunk_sh)

    bank_of_chunk = (2 * chunk_w) // 8
    start_f = np.zeros(NCHUNK, bool); stop_f = np.zeros(NCHUNK, bool)
    for b in np.unique(bank_of_chunk):
        idxs = np.nonzero(bank_of_chunk == b)[0]
        start_f[idxs[0]] = True; stop_f[idxs[-1]] = True

    calls = []
    for sh, lo, hi in seg_bounds:
        c0 = lo
        while c0 < hi:
            c1 = min(c0 + CALLCH, hi)
            calls.append((sh, c0, c1))
            c0 = c1
    grp_of_chunk = (2 * chunk_w) // WGRP

    # --- per-core edge payloads --------------------------------------------
    idx_all = np.zeros((C, NCHUNK * 128), np.int16)
    dl_all = np.full((C, NCHUNK * 128), 400.0, np.float32)
    for c in range(C):
        m = ecore == c
        es, ish, iw, idl = src[m], esh[m], ewp[m], edloc[m]
        order = np.lexsort((srow[es], iw, ish))
        es, ish, iw, idl = es[order], ish[order], iw[order], idl[order]
        key = ish * WP + iw
        uniq, first = np.unique(key, return_index=True)
        ranks = np.arange(len(key)) - first[np.searchsorted(uniq, key)]
        pos = boff[ish, iw] * 128 + ranks
        idx_all[c, pos] = (srow[es] - ish * HR).astype(np.int16)
        dl_all[c, pos] = idl

    idx16 = np.zeros((C, 128, NCHUNK * 8), np.int16)
    for c in range(C):
        wrapped = idx_all[c].reshape(NCHUNK * 8, 16).T
        idx16[c] = np.tile(wrapped, (8, 1))
    dstloc = np.zeros((C, 128, NCHUNK), NP_BF16)
    for c in range(C):
        dstloc[c] = dl_all[c].reshape(NCHUNK, 128).T.astype(NP_BF16)

    # --- node-side tensors --------------------------------------------------
    xfm = np.zeros((F, ROWS), np.float32)
    xfm[:, xcol] = x.T
    xfm = xfm.astype(NP_BF16)

    disn = np.zeros((C, 128, W), np.float32)
    for c in range(C):
        sl = slice(c * NPC, (c + 1) * NPC)
        disn[c][p_[sl], w_[sl]] = dis[sl]

    tailp = NPC - (W - 1) * 128
    mask48 = (np.arange(128) < tailp).astype(np.float32).reshape(128, 1)

    BLK = math.ceil(G / 128) + 3
    wk = [int(batch[c * NPC]) // 128 for c in range(C)]
    glocal = np.full((C, 128, W), 1.0e4, np.float32)
    for c in range(C):
        sl = slice(c * NPC, (c + 1) * NPC)
        gl = batch[sl] - 128 * wk[c]
        assert gl.min() >= 0 and gl.max() < 384, (c, gl.min(), gl.max())
        glocal[c][p_[sl], w_[sl]] = gl

    meta = dict(
        N=N, F=F, H=H, E=E, G=G, C=C, NPC=NPC, W=W, NPAD=NPAD, ROWS=ROWS,
        HR=HR, NG=NG, nbs=nbs, cum_nb=cum_nb, base_g=base_g,
        NCHUNK=NCHUNK, chunk_w=chunk_w, chunk_sh=chunk_sh,
        bank_of_chunk=bank_of_chunk,
        start_f=start_f, stop_f=stop_f, calls=calls, grp_of_chunk=grp_of_chunk,
        wk=wk, BLK=BLK, tailp=tailp,
    )

    shared = dict(
        W1sb=W1.astype(NP_BF16),
        W2sb=W2.astype(NP_BF16),
        b1sb=np.tile(b1.reshape(1, H), (128, 1)).astype(np.float32),
        b2sb=np.tile(b2.reshape(1, H), (128, 1)).astype(np.float32),
        iota128=np.tile(np.arange(256, dtype=np.float32).reshape(1, 256),
                        (128, 1)).astype(NP_BF16),
        iotaP=np.tile(np.arange(384, dtype=np.float32).reshape(1, 384),
                      (128, 1)),
        mask48=mask48,
    )
    in_maps = []
    for c in range(C):
        m = dict(shared)
        m["xown"] = np.ascontiguousarray(xfm[:, c * NPAD:(c + 1) * NPAD])
        m["idx16"] = idx16[c]
        m["dstloc"] = dstloc[c]
        m["disn"] = disn[c]
        m["glocal"] = glocal[c]
        in_maps.append(m)
    return meta, in_maps


def build(nc, meta):
    F, H, C = meta["F"], meta["H"], meta["C"]
    W, NPAD, ROWS, HR = meta["W"], meta["NPAD"], meta["ROWS"], meta["HR"]
    NG, NCHUNK, BLK, G = meta["NG"], meta["NCHUNK"], meta["BLK"], meta["G"]
    nbs, cum_nb, base_g = meta["nbs"], meta["cum_nb"], meta["base_g"]
    chunk_w = meta["chunk_w"]
    start_f, stop_f = meta["start_f"], meta["stop_f"]
    calls, wk = meta["calls"], meta["wk"]
    rg = [list(range(C))]

    W1_e = nc.dram_tensor("W1sb", [F, H], BF16, kind="ExternalInput")
    W2_e = nc.dram_tensor("W2sb", [H, H], BF16, kind="ExternalInput")
    b1_e = nc.dram_tensor("b1sb", [128, H], F32, kind="ExternalInput")
    b2_e = nc.dram_tensor("b2sb", [128, H], F32, kind="ExternalInput")
    iota_e = nc.dram_tensor("iota128", [128, 256], BF16, kind="ExternalInput")
    iotaP_e = nc.dram_tensor("iotaP", [128, 384], F32, kind="ExternalInput")
    mask_e = nc.dram_tensor("mask48", [128, 1], F32, kind="ExternalInput")
    idx_e = nc.dram_tensor("idx16", [128, NCHUNK * 8], I16, kind="ExternalInput")
    dstloc_e = nc.dram_tensor("dstloc", [128, NCHUNK], BF16, kind="ExternalInput")
    disn_e = nc.dram_tensor("disn", [128, W], F32, kind="ExternalInput")
    glocal_e = nc.dram_tensor("glocal", [128, W], F32, kind="ExternalInput")
    xown_e = nc.dram_tensor("xown", [F, NPAD], BF16, kind="ExternalInput")
    out_e = nc.dram_tensor("out", [128, 3 * H], F32, kind="ExternalOutput")

    PW = 3 * H
    shared = "Shared" if C > 4 else "Local"
    tblA = nc.dram_tensor("tblA", [ROWS, 128], BF16, addr_space=shared)
    tblB = nc.dram_tensor("tblB", [ROWS, 128], BF16, addr_space=shared)
    own_slice = nc.dram_tensor("own_slice", [NPAD, 128], BF16)

    with tile.TileContext(nc) as tc:
        from contextlib import ExitStack
        with ExitStack() as ctx:
            cpool = ctx.enter_context(tc.tile_pool(name="const", bufs=1))
            spool = ctx.enter_context(tc.tile_pool(name="s", bufs=8))
            mpool = ctx.enter_context(tc.tile_pool(name="msg", bufs=3))
            tpool = ctx.enter_context(tc.tile_pool(name="tmp", bufs=3))
            agg_ps = ctx.enter_context(
                tc.tile_pool(name="aggps", bufs=2, space="PSUM"))
            tp_ps = ctx.enter_context(
                tc.tile_pool(name="tpps", bufs=2, space="PSUM"))

            def load(name, ext, shape, dt):
                t = cpool.tile(shape, dt, tag=name)
                nc.sync.dma_start(t[:], ext.ap())
                return t

            W1sb = load("W1", W1_e, [F, H], BF16)
            disn = load("disn", disn_e, [128, W], F32)
            W2sb = load("W2", W2_e, [H, H], BF16)
            b1sb = load("b1", b1_e, [128, H], F32)
            b2sb = load("b2", b2_e, [128, H], F32)
            iota = load("iota", iota_e, [128, 256], BF16)
            iotaP = load("iotaP", iotaP_e, [128, 384], F32)
            mask48 = load("mask48", mask_e, [128, 1], F32)
            idxsb = load("idx", idx_e, [128, NCHUNK * 8], I16)
            dstloc = load("dstloc", dstloc_e, [128, NCHUNK], BF16)
            glocal = load("glocal", glocal_e, [128, W], F32)
            ident = cpool.tile([128, 128], BF16, tag="ident")
            make_identity(nc, ident[:])

            h_fm = [cpool.tile([H, NPAD], BF16, tag=f"hfm{i}", name=f"hfm{i}")
                    for i in range(2)]
            t_own = cpool.tile([128, W * H], BF16, tag="town")
            tpad = cpool.tile([128, W * 128], BF16, tag="tpad")
            nc.vector.memset(tpad[:], 0.0)
            h_nm = cpool.tile([128, W * H], BF16, tag="hnm")

            def iota_bc(nb, width=256, base=0):
                a = iota[:]
                return _ap3(a, [[a.ap[0][0], 128], [0, nb], [1, width]], base)

            def sc_bc(t, lo, n, inner):
                a = t[:]
                step = a.ap[0][0]
                return _ap3(a, [[step, 128], [1, n], [0, inner]], a.offset + lo)

            def transform_own_grp(h_src, Wsb, g):
                wlo = g * WGRP
                nb = nbs[g]
                ps = agg_ps.tile([128, WGRP * H], F32, tag="agg",
                                 name=f"tf{g}")
                for i in range(nb):
                    w = wlo + i
                    nc.tensor.matmul(
                        ps[:, i * H:(i + 1) * H],
                        lhsT=h_src[:, w * 128:(w + 1) * 128],
                        rhs=Wsb[:],
                        start=True, stop=True, skip_group_check=True)
                ps3 = _ap3(ps[:], [[ps[:].ap[0][0], 128], [H, nb], [1, H]])
                t3 = _ap3(t_own[:], [[t_own[:].ap[0][0], 128], [H, nb], [1, H]],
                          t_own[:].offset + wlo * H)
                nc.vector.tensor_tensor(
                    t3, ps3, sc_bc(disn, wlo, nb, H), op=ALU.mult)
                tp3 = _ap3(tpad[:], [[tpad[:].ap[0][0], 128], [128, nb], [1, H]],
                           tpad[:].offset + wlo * 128)
                to3 = _ap3(t_own[:], [[t_own[:].ap[0][0], 128], [H, nb], [1, H]],
                           t_own[:].offset + wlo * H)
                nc.vector.tensor_copy(tp3, to3)

            pending_cc = []

            def exchange_grp(g, tbl, defer=False):
                nb = nbs[g]
                wlo = g * WGRP
                osl = _ap3(own_slice.ap(),
                           [[nb * 128, 128], [1, nb * 128]],
                           int(cum_nb[g]) * 128 * 128)
                nc.sync.dma_start(osl, tpad[:, wlo * 128:(wlo + nb) * 128])
                ins = bass.AP(own_slice.ap().tensor,
                              int(cum_nb[g]) * 128 * 128,
                              [[1, nb * 128 * 128]])
                outs = bass.AP(tbl.ap().tensor, base_g[g] * 128,
                               [[1, C * nb * 128 * 128]])
                if defer:
                    pending_cc.append((ins, outs))
                    return
                nc.gpsimd.collective_compute(
                    "AllGather", ALU.bypass, replica_groups=rg,
                    ins=[ins.opt()], outs=[outs.opt()])

            def flush_cc():
                while pending_cc:
                    ins, outs = pending_cc.pop(0)
                    nc.gpsimd.collective_compute(
                        "AllGather", ALU.bypass, replica_groups=rg,
                        ins=[ins.opt()], outs=[outs.opt()])

            def aggregate_and_drain(layer, bsb, tbl, after_group=None):
                psg = {}
                bank_start = {}
                glast = {}
                for ci in range(NCHUNK):
                    glast[int(meta["grp_of_chunk"][ci])] = ci
                drain_after = {v: k for k, v in glast.items()}
                for ci_call, (sh, c_lo, c_hi) in enumerate(calls):
                    if ci_call >= 1:
                        flush_cc()
                    ncall = c_hi - c_lo
                    msg = mpool.tile([128, CALLCH * 128], BF16, tag="msg")
                    in_ap = bass.AP(tbl.ap().tensor, sh * HR * 128,
                                    [[128, HR], [1, 128]])
                    nc.gpsimd.dma_gather(
                        out_ap=_ap3(msg[:], [[msg[:].ap[0][0], 128],
                                             [128, ncall], [1, 128]]),
                        in_ap=in_ap,
                        idxs_ap=idxsb[:, c_lo * 8:c_hi * 8],
                        num_idxs=ncall * 128,
                        num_idxs_reg=ncall * 128,
                        elem_size=128,
                        single_packet=(ncall * 128 <= 1024))
                    for s0 in range(0, ncall, SBATCH):
                        nb_s = min(SBATCH, ncall - s0)
                        S = spool.tile([128, SBATCH * 256], BF16, tag="S")
                        S3 = _ap3(S[:], [[S[:].ap[0][0], 128], [256, nb_s],
                                         [1, 256]])
                        nc.vector.tensor_tensor(
                            S3, iota_bc(nb_s),
                            sc_bc(dstloc, c_lo + s0, nb_s, 256),
                            op=ALU.is_equal)
                        for j in range(nb_s):
                            ci = c_lo + s0 + j
                            wp = int(chunk_w[ci])
                            wA = 2 * wp
                            g = wA // WGRP
                            if g not in psg:
                                psg[g] = agg_ps.tile(
                                    [128, WGRP * H], F32, tag="agg",
                                    name=f"agg_l{layer}_g{g}")
                            wl = wA - g * WGRP
                            has_b = wA + 1 < W
                            rhs = msg[:, (s0 + j) * 128:(s0 + j) * 128 + H]
                            mm = nc.tensor.matmul(
                                psg[g][:, wl * H:(wl + 1) * H],
                                lhsT=S[:, j * 256:j * 256 + 128],
                                rhs=rhs,
                                start=bool(start_f[ci]),
                                stop=bool(stop_f[ci]) and not has_b,
                                skip_group_check=True)
                            bk = int(meta["bank_of_chunk"][ci])
                            if start_f[ci]:
                                bank_start[bk] = mm
                            elif bk in bank_start:
                                bass._add_dep_helper(
                                    mm.ins, bank_start[bk].ins, sync=False,
                                    reason="psum zero-region order")
                            if has_b:
                                mmb = nc.tensor.matmul(
                                    psg[g][:, (wl + 1) * H:(wl + 2) * H],
                                    lhsT=S[:, j * 256 + 128:(j + 1) * 256],
                                    rhs=rhs,
                                    start=False, stop=bool(stop_f[ci]),
                                    skip_group_check=True)
                                bass._add_dep_helper(
                                    mmb.ins, bank_start[bk].ins, sync=False,
                                    reason="psum zero-region order")
                            if ci in drain_after:
                                gdone = drain_after[ci]
                                drain_group(gdone, psg.pop(gdone), bsb)
                                if after_group is not None:
                                    after_group(gdone)

            def drain_group(g, ps, bsb):
                wlo = g * WGRP
                nb = nbs[g]
                pstep = ps[:].ap[0][0]
                ps3 = _ap3(ps[:], [[pstep, 128], [H, nb], [1, H]])
                tmp = tpool.tile([128, WGRP * H], F32, tag="dr")
                ts = tmp[:].ap[0][0]
                tmp3 = _ap3(tmp[:], [[ts, 128], [H, nb], [1, H]])
                to3 = _ap3(t_own[:], [[t_own[:].ap[0][0], 128], [H, nb], [1, H]],
                           t_own[:].offset + wlo * H)
                nc.vector.tensor_tensor(tmp3, ps3, to3, op=ALU.add)
                nc.vector.tensor_tensor(
                    tmp3, tmp3, sc_bc(disn, wlo, nb, H), op=ALU.mult)
                bb = _ap3(bsb[:], [[bsb[:].ap[0][0], 128], [0, nb], [1, H]], 0)
                nc.vector.tensor_tensor(tmp3, tmp3, bb, op=ALU.add)
                hn3 = _ap3(h_nm[:], [[h_nm[:].ap[0][0], 128], [H, nb], [1, H]],
                           h_nm[:].offset + wlo * H)
                nc.vector.tensor_scalar(hn3, tmp3, 0.0, None, op0=ALU.max)
                if g == NG - 1 and meta["tailp"] < 128:
                    lastw = W - 1
                    hl = _ap3(h_nm[:], [[h_nm[:].ap[0][0], 128], [1, H]],
                              h_nm[:].offset + lastw * H)
                    mb = _ap3(mask48[:],
                              [[mask48[:].ap[0][0], 128], [0, H]], 0)
                    nc.vector.tensor_tensor(hl, hl, mb, op=ALU.mult)

            def to_fm_grp(dst_fm, g):
                for w in range(g * WGRP, g * WGRP + nbs[g]):
                    tp = tp_ps.tile([H, 128], BF16, tag="tp")
                    nc.tensor.transpose(
                        out=tp[:],
                        in_=_ap3(h_nm[:], [[h_nm[:].ap[0][0], 128], [1, H]],
                                 h_nm[:].offset + w * H),
                        identity=ident[:])
                    nc.scalar.copy(dst_fm[:, w * 128:(w + 1) * 128], tp[:])

            # ================= layer schedule =================
            tbls = [tblA, tblB]
            with tc.tile_pool(name="xfm", bufs=2) as xpool:
                xo = xpool.tile([F, NPAD], BF16, tag="xfm", name="xo")
                nc.sync.dma_start(xo[:], xown_e.ap())
                for g in range(NG):
                    transform_own_grp(xo, W1sb, g)
                    exchange_grp(g, tbls[0])

            pool_state = {}

            def pooling_grp(g):
                wlo = g * WGRP
                nwg = nbs[g]
                pps = pool_state["pps"]
                for blk in range(3):
                    for w0 in range(wlo, wlo + nwg, SBATCH):
                        nb = min(SBATCH, wlo + nwg - w0)
                        SG = spool.tile([128, SBATCH * 256], BF16, tag="S")
                        iob = _ap3(iotaP[:], [[iotaP[:].ap[0][0], 128],
                                              [0, nb], [1, 128]], blk * 128)
                        nc.vector.tensor_tensor(
                            _ap3(SG[:], [[SG[:].ap[0][0], 128], [128, nb],
                                         [1, 128]]),
                            iob, sc_bc(glocal, w0, nb, 128), op=ALU.is_equal)
                        for i in range(nb):
                            w = w0 + i
                            mm = nc.tensor.matmul(
                                pps[:, blk * H:(blk + 1) * H],
                                lhsT=SG[:, i * 128:(i + 1) * 128],
                                rhs=_ap3(h_nm[:],
                                         [[h_nm[:].ap[0][0], 128], [1, H]],
                                         h_nm[:].offset + w * H),
                                start=(blk == 0 and w == 0),
                                stop=(blk == 2 and w == W - 1),
                                skip_group_check=True)
                            if blk == 0 and w == 0:
                                pool_state["start"] = mm
                            else:
                                bass._add_dep_helper(
                                    mm.ins, pool_state["start"].ins,
                                    sync=False,
                                    reason="psum zero-region order")

            for l in range(1, 6):
                bsb = b1sb if l == 1 else b2sb
                tbl_r = tbls[(l - 1) % 2]
                tbl_w = tbls[l % 2]
                if l < 5:
                    hf_next = h_fm[(l + 1) % 2]

                    def after_group(g, hf=hf_next, tw=tbl_w):
                        to_fm_grp(hf, g)
                        transform_own_grp(hf, W2sb, g)
                        exchange_grp(g, tw, defer=True)
                else:
                    pool_state["pps"] = agg_ps.tile(
                        [128, WGRP * H], F32, tag="agg", name="pps")

                    def after_group(g):
                        pooling_grp(g)
                aggregate_and_drain(l, bsb, tbl_r, after_group=after_group)

            flush_cc()
            ppsb = tpool.tile([128, PW], F32, tag="ppsb")
            nc.vector.tensor_copy(ppsb[:], pool_state["pps"][:, :PW])
            nc.sync.dma_start(out_e.ap(), ppsb[:])


def run(inputs, C=8, G=1000, trace=False):
    meta, in_maps = prep(
        inputs["x"], inputs["W1"], inputs["b1"], inputs["W2"], inputs["b2"],
        inputs["Wl"], inputs["bl"], inputs["edge_index"], inputs["batch"],
        C=C, G=G)
    nc = bacc.Bacc("TRN2", target_bir_lowering=False, debug=False,
                   num_devices=C)
    build(nc, meta)
    nc.compile()
    from concourse.bass_utils import run_bass_kernel_spmd
    res = run_bass_kernel_spmd(nc, in_maps, core_ids=list(range(C)),
                               trace=trace)
    parts = [res.results[c]["out"] for c in range(C)]
    out = host_finish(meta, parts, inputs, C, G)
    return out, res


def host_finish(meta, parts, inputs, C, G):
    H = meta["H"]
    pooled = np.zeros(((meta["BLK"] + 3) * 128, H), np.float32)
    for c in range(C):
        part = np.asarray(parts[c], np.float32)
        base = meta["wk"][c] * 128
        for b in range(3):
            pooled[base + b * 128: base + (b + 1) * 128] += \
                part[:, b * H:(b + 1) * H]
    counts = np.bincount(np.asarray(inputs["batch"], np.int64),
                         minlength=G).astype(np.float32)
    pooledG = pooled[:G] / np.maximum(counts, 1.0)[:, None]
    Wl = np.asarray(inputs["Wl"], np.float32).reshape(H, -1)
    bl = np.asarray(inputs["bl"], np.float32)
    return (pooledG @ Wl + bl).astype(np.float32)


def kernel(**inputs):
    out, _ = run(inputs)
    return out
